# revision 84
# baseline (speedup 1.0000x reference)
"""AdaFace loss kernel for 8 TRN2 NeuronCores (raw Bass, hand-scheduled).

Sharding: class dimension (C=100000) split across 8 cores -> [1024, 12500]
shard per core (partial-FC / vocab parallel); labels/norms replicated.

Math: for logits x in (-0.99, 0.99), arccos(x) lies strictly inside
[eps, pi-eps], so cos(clip(arccos(x), eps, pi-eps)) == x for every column
except the (row, label) entry of positive rows.  Hence

    out = 64 * x                 everywhere, plus
    out[r, l_r] = 64 * (cos(clip(arccos(x_rl) + g_ang_r, eps, pi-eps)) - g_add_r)

The bulk stream rides fp16 (the 2e-2 rel-err budget dwarfs fp16's ~1e-4):
logits are quantized to fp16 on host, streamed in, scaled by 64 and the
per-row one-hot delta added, and written back as fp16.  HBM traffic is
4 B/elem instead of 8, and the 8-core aggregate sits at the chip HBM
wall (~3 TB/s).  Streamed DVE ops are chosen for the fast 16-bit modes
(measured on HW): tensor_scalar runs 2x for the x*64 scale and the
(ramp==loc)*delta one-hot build (int16 ramp / fp16 out; scalar [P,1]
operands may stay f32); tensor_tensor add runs 1x; scalar_tensor_tensor
(2 cycles/elem, no fast mode) is avoided in the stream.

delta_r = 64*(v_r - x_rl) with x_rl read from the QUANTIZED logits so the
delta cancels the bulk term exactly.  The AdaFace margin statistics
(mean/unbiased-std of clipped feature norms over positive rows) are
computed on device with DVE free-dim reductions + a PE ones-matmul for the
partition-dim reduce-and-broadcast.

cos(theta+g) is evaluated without arccos via the identity
    cos(arccos(x)+g) = x*cos(g) - sqrt(1-x^2)*sin(g)
and the theta-space clip maps to x-space threshold tests:
    theta+g < eps      <=>  (g <= eps)  and  x > cos(eps-g)
    theta+g > pi-eps   <=>  (g >= -eps) and  x < -cos(eps+g)

Loads ride the gpsimd SWDGE queue; stores ride the SP hardware DGE.
Splitting the two dispatch paths balances the 16 SDMA engines (a single
SWDGE stream systematically starves engine 15 by ~15%) and removes the
store descriptor-generation latency.  The stream is a hand-rolled
double-buffered pipeline with per-slot semaphores, so every instruction
carries at most ONE sync wait (this walrus build rejects more).  Tiles
are a full row-block wide (T=12500 -> 25 KB per-partition DMA
descriptors); the stats prologue takes one DVE->PE->DVE and one
DVE->ACT->DVE roundtrip (1/(std+eps) ~= sqrt(1/var) on ACT, rel diff
~1e-4).
"""

import math
import sys
from contextlib import ExitStack

import numpy as np

sys.path.insert(0, "/opt/trn_rl_repo")

# ---- problem constants (hardcoded per instructions) ----
B = 1024
C = 100000
NCORES = 8
CSH = C // NCORES          # 12500 columns per core
NSH = B * CSH              # flat shard length
P = 128                    # partitions
RB = B // P                # 8 row blocks
T = CSH                    # free-dim tile: full shard row (25KB f16 descriptors)
NTILES = RB                # 8 stream tiles
XB = 3                     # x-tile buffers (prefetch depth)
DB = 3                     # d-tile buffers (decouple pass1 from stores)
M_C = 0.4
EPS = 1e-3
S = 64.0
COS_EPS = math.cos(EPS)
PI = math.pi

_CACHED = {}


# load units: (rb, off, w) — what each DMA load brings into an x-buffer
# (last tile split to shrink the tail; slot = index % XB)
LOADS = [(k, 0, T) for k in range(NTILES - 1)]
LOADS += [(RB - 1, 0, T // 2), (RB - 1, T // 2, T // 2)]
NL = len(LOADS)

# compute/store units: (load_idx, boff, w) — 1:1 with loads
UNITS = [(l, 0, LOADS[l][2]) for l in range(NL)]
NU = len(UNITS)

# gp load l (l>=XB) reuses the slot of load l-XB; it may only start after
# the LAST compute unit reading that buffer has retired (sC value).
_last_compute_of_load = {}
for _c, (_l, _b, _w) in enumerate(UNITS):
    _last_compute_of_load[_l] = _c
LOAD_GATE = {l: _last_compute_of_load[l - XB] + 1 for l in range(XB, NL)}


def _build_program():
    import concourse.bass as bass
    from concourse import mybir

    f32 = mybir.dt.float32
    f16 = mybir.dt.float16
    i16 = mybir.dt.int16
    u32 = mybir.dt.uint32
    Alu = mybir.AluOpType
    Act = mybir.ActivationFunctionType
    AxX = mybir.AxisListType.X

    nc = bass.Bass()

    lg = nc.declare_dram_parameter("logits", [NSH], f16, isOutput=False)
    # packed sidecar: [0:8]=norms [8:16]=posf [16:24]=mmask [24:32]=locf
    # [32:40]=xv (quantized logits at label columns, replicated)
    sdc = nc.declare_dram_parameter("sidecar", [P, 5 * RB], f32, isOutput=False)
    rmp = nc.declare_dram_parameter("ramp", [P, T], i16, isOutput=False)
    out = nc.declare_dram_parameter("out", [NSH], f16, isOutput=True)

    lg2d = lg[:].rearrange("(a b) -> a b", b=CSH)
    out2d = out[:].rearrange("(a b) -> a b", b=CSH)

    def loadslice(dram2d, l):
        rb, off, w = LOADS[l]
        return dram2d[rb * P : (rb + 1) * P, off : off + w]

    def unitgeom(c):
        # -> (load_idx, slot, boff, goff, w, rb) for compute unit c
        l, boff, w = UNITS[c]
        rb, loff, _ = LOADS[l]
        return l, l % XB, boff, loff + boff, w, rb

    ctx = ExitStack()

    def sb(name, shape, dtype=f32):
        return ctx.enter_context(nc.sbuf_tensor(name, shape, dtype))[:]

    def psb(name, shape):
        return ctx.enter_context(nc.psum_tensor(name, shape, f32))[:]

    def sem(name):
        return ctx.enter_context(nc.semaphore(name))

    with ctx:
        sd = sb("sd", [P, 5 * RB])
        ramp = sb("ramp_t", [P, T], i16)
        xt = [sb(f"x{i}", [P, T], f16) for i in range(XB)]
        dt = [sb(f"d{i}", [P, T], f16) for i in range(DB)]
        ones = sb("ones", [P, P])
        sn = sb("sn", [P, RB]); snp = sb("snp", [P, RB])
        sn2p = sb("sn2p", [P, RB]); red1 = sb("red1", [P, 3])
        tot1 = sb("tot1", [P, 3]); rc = sb("rc", [P, 1]); mean = sb("mean", [P, 1])
        dev = sb("dev", [P, RB]); sm = sb("sm", [P, 1]); vnum = sb("vnum", [P, 1])
        cm1 = sb("cm1", [P, 1])
        rcm1 = sb("rcm1", [P, 1]); var = sb("var", [P, 1])
        rvar = sb("rvar", [P, 1])
        rstd = sb("rstd", [P, 1]); ms = sb("ms", [P, RB])
        gadd = sb("gadd", [P, RB])
        b_hpi = sb("b_hpi", [P, 1]); b_hpe = sb("b_hpe", [P, 1])
        b_nhpe = sb("b_nhpe", [P, 1])
        cg = sb("cg", [P, RB]); sg = sb("sg", [P, RB])
        x2 = sb("xvsq", [P, RB]); sq = sb("sq", [P, RB])
        t1 = sb("t1", [P, RB]); t2 = sb("t2", [P, RB]); tt = sb("tt", [P, RB])
        negu = sb("negu", [P, RB]); cb = sb("cb", [P, RB])
        chi = sb("chi", [P, RB], u32); u2 = sb("u2", [P, RB])
        cc = sb("cc", [P, RB])
        clo = sb("clo", [P, RB], u32)
        negc = sb("negc", [P, RB]); posc = sb("posc", [P, RB])
        vfin = sb("vfin", [P, RB])
        dvx = sb("dvx", [P, RB])
        delta = sb("delta", [P, RB])
        ps1 = psb("ps1", [P, 3])

        nrm_t = sd[:, 0 * RB : 1 * RB]
        pos_t = sd[:, 1 * RB : 2 * RB]
        m_t = sd[:, 2 * RB : 3 * RB]
        loc_t = sd[:, 3 * RB : 4 * RB]
        xvv = sd[:, 4 * RB : 5 * RB]

        # NOTE: DMA sems count per-SDMA-engine increments (16 per DMA).
        # With >1 DMA in flight on one sem, partial completions of later
        # DMAs can satisfy an earlier wait -> per-SLOT sems so each sem
        # has at most one DMA outstanding (slot reuse serializes them).
        dS = sem("sidecar_dma")
        dR = sem("ramp_dma")
        sLs = [sem(f"load{i}") for i in range(XB)]
        sSs = [sem(f"store{i}") for i in range(DB)]
        sC = sem("compute")  # per-tile TT done (+1 each) -> store k
        hDP = sem("dve2pe")
        hPD = sem("pe2dve")
        hDA = sem("dve2act")
        hAD = sem("act2dve")

        with nc.Block() as block:

            @block.gpsimd
            def _(gp):
                gp.dma_start(out=sd, in_=sdc[:]).then_inc(dS, 16)
                gp.dma_start(out=xt[0][:, 0 : LOADS[0][2]], in_=loadslice(lg2d, 0)).then_inc(sLs[0], 16)
                gp.dma_start(out=ramp, in_=rmp[:]).then_inc(dR, 16)
                for l in range(1, XB):
                    gp.dma_start(
                        out=xt[l % XB][:, 0 : LOADS[l][2]], in_=loadslice(lg2d, l)
                    ).then_inc(sLs[l % XB], 16)
                for l in range(XB, NL):
                    gp.wait_ge(sC, LOAD_GATE[l])
                    gp.dma_start(
                        out=xt[l % XB][:, 0 : LOADS[l][2]], in_=loadslice(lg2d, l)
                    ).then_inc(sLs[l % XB], 16)

            @block.sync
            def _(sp):
                # stores ride the SP hardware DGE: descriptor gen in HW,
                # decoupled from the gpsimd load dispatch stream
                for c in range(NU):
                    _, slot, boff, goff, w, rb = unitgeom(c)
                    sp.wait_ge(sC, c + 1)
                    sp.dma_start(
                        out=out2d[rb * P : (rb + 1) * P, goff : goff + w],
                        in_=dt[c % DB][:, 0:w],
                    ).then_inc(sSs[c % DB], 16)
                for i in range(DB):
                    sp.wait_ge(sSs[i], 16 * len([c for c in range(NU) if c % DB == i]))

            @block.vector
            def _(v):
                v.memset(ones, 1.0)
                v.memset(b_hpi, PI / 2)
                v.memset(b_hpe, PI / 2 + EPS)
                v.memset(b_nhpe, -PI / 2 - EPS)
                v.memset(negc, -COS_EPS)
                v.memset(posc, COS_EPS)

                def xs(c):
                    # scale unit c's x-slice by S in place (16-bit 2x TS)
                    l, slot, boff, goff, w, rb = unitgeom(c)
                    v.wait_ge(sLs[l % XB], 16 * (l // XB + 1))
                    v.tensor_scalar(
                        xt[slot][:, boff : boff + w],
                        xt[slot][:, boff : boff + w],
                        S,
                        None,
                        Alu.mult,
                    )

                v.wait_ge(dS, 16)
                # stats round 1: sums of sn*p, p, sn^2*p (one PE reduction)
                v.tensor_scalar(sn, nrm_t, 1e-3, 100.0, Alu.max, Alu.min)
                v.drain()
                v.tensor_tensor(snp, sn, pos_t, Alu.mult)
                v.drain()
                v.tensor_tensor(sn2p, snp, sn, Alu.mult)
                v.tensor_reduce(red1[:, 0:1], snp, axis=AxX, op=Alu.add)
                v.tensor_reduce(red1[:, 1:2], pos_t, axis=AxX, op=Alu.add)
                v.drain()
                v.tensor_reduce(red1[:, 2:3], sn2p, axis=AxX, op=Alu.add)
                v.drain().then_inc(hDP, 1)
                v.wait_ge(hPD, 1)
                v.tensor_copy(tot1, ps1)
                v.drain()
                v.reciprocal(rc, tot1[:, 1:2])
                v.tensor_scalar_add(cm1, tot1[:, 1:2], -1.0)
                v.drain()
                v.tensor_tensor(mean, tot1[:, 0:1], rc, Alu.mult)
                v.reciprocal(rcm1, cm1)
                v.drain()
                # var = (s2 - s1*mean) / (cnt-1)
                v.tensor_tensor(sm, tot1[:, 0:1], mean, Alu.mult)
                v.tensor_scalar(dev, sn, mean, None, Alu.subtract)
                v.drain()
                v.tensor_tensor(vnum, tot1[:, 2:3], sm, Alu.subtract)
                v.drain()
                v.tensor_tensor(var, vnum, rcm1, Alu.mult)
                v.drain()
                v.reciprocal(rvar, var)
                v.drain().then_inc(hDA, 1)
                v.wait_ge(hAD, 1)
                # gadd = M + M*ms ; independent group then combine
                v.tensor_scalar(gadd, ms, M_C, M_C, Alu.mult, Alu.add)
                v.tensor_tensor(t1, xvv, cg, Alu.mult)
                v.tensor_tensor(t2, sq, sg, Alu.mult)
                v.tensor_tensor(cb, xvv, negu, Alu.is_lt)
                v.tensor_tensor(cc, xvv, u2, Alu.is_gt)
                v.drain()
                v.tensor_tensor(tt, t1, t2, Alu.subtract)
                # chi = (ms <= eps/M) & (xv < -cos(g+eps))
                v.scalar_tensor_tensor(chi, ms, EPS / M_C, cb, Alu.is_le, Alu.mult)
                # clo = (ms >= -eps/M) & (xv > cos(eps-g))
                v.scalar_tensor_tensor(clo, ms, -EPS / M_C, cc, Alu.is_ge, Alu.mult)
                v.drain()
                v.copy_predicated(tt, chi, negc)
                v.drain()
                v.copy_predicated(tt, clo, posc)
                v.drain()
                v.tensor_tensor(vfin, tt, gadd, Alu.subtract)
                v.drain()
                v.tensor_tensor(dvx, vfin, xvv, Alu.subtract)
                v.drain()
                # delta = S * (vfin - xv) * mmask   (S folded here, not in stream)
                v.scalar_tensor_tensor(delta, dvx, S, m_t, Alu.mult, Alu.mult)
                v.drain()

                def pass1(c):
                    _, slot, boff, goff, w, rb = unitgeom(c)
                    v.tensor_scalar(
                        dt[c % DB][:, 0:w],
                        ramp[:, goff : goff + w],
                        loc_t[:, rb : rb + 1],
                        delta[:, rb : rb + 1],
                        Alu.is_equal,
                        Alu.mult,
                    )

                # stream: per unit k
                #   TS  xt_k *= 64          (16-bit 2x, in-place)
                #   TS  pass1(k+1) -> dt    (one-hot build, 16-bit 2x)
                #   drain -> sC2  (flushes both, and pass1(k) from prev iter)
                #   TT  dt_k = xt_k + dt_k  (1x)
                #   drain -> sC
                v.wait_ge(dR, 16)
                pass1(0)
                for c in range(NU):
                    _, slot, boff, goff, w, rb = unitgeom(c)
                    xs(c)
                    if c + 1 < NU:
                        j = c + 1
                        if j >= DB:
                            v.wait_ge(sSs[j % DB], 16 * (j // DB))
                        pass1(j)
                    v.drain()
                    v.tensor_tensor(
                        dt[c % DB][:, 0:w],
                        xt[slot][:, boff : boff + w],
                        dt[c % DB][:, 0:w],
                        Alu.add,
                    )
                    v.drain().then_inc(sC, 1)

            @block.scalar
            def _(sc):
                sc.wait_ge(dS, 16)
                sc.activation(x2, xvv, Act.Square)
                sc.drain()
                sc.activation(sq, x2, Act.Sqrt, scale=-1.0, bias=1.0)
                sc.wait_ge(hDA, 1)
                # 1/(std+EPS) ~= sqrt(1/var): rel diff EPS/std ~1e-4, within
                # budget; saves a DVE->ACT->DVE roundtrip + 2 ops
                sc.activation(rstd, rvar, Act.Sqrt)
                sc.drain()
                # ms = dev * rstd via per-partition activation scale
                sc.activation(ms, dev, Act.Identity, scale=rstd[:, 0:1])
                sc.drain()
                # g = -M*ms folded into the activation scale
                sc.activation(cg, ms, Act.Sin, scale=-M_C, bias=b_hpi)
                sc.activation(sg, ms, Act.Sin, scale=-M_C)
                sc.activation(negu, ms, Act.Sin, scale=M_C, bias=b_nhpe)
                sc.activation(u2, ms, Act.Sin, scale=M_C, bias=b_hpe)
                sc.drain().then_inc(hAD, 1)

            @block.tensor
            def _(te):
                te.wait_ge(hDP, 1)
                te.matmul(ps1, lhsT=ones, rhs=red1, start=True, stop=True)
                te.drain().then_inc(hPD, 1)

    return nc


def _get_program():
    if "nc" not in _CACHED:
        _CACHED["nc"] = _build_program()
    return _CACHED["nc"]


def _prep_inputs(logits, norms, labels):
    """Shard across 8 cores; build per-core index/mask sidecar tensors."""
    labels = np.asarray(labels).astype(np.int64)
    logits = np.asarray(logits, dtype=np.float32)
    norms = np.asarray(norms, dtype=np.float32)

    rows = np.arange(B, dtype=np.int64)
    posf = (labels >= 0).astype(np.float32)

    def fold(a):
        # [B] -> [P, RB] with element (p, rb) = row rb*P + p
        return np.ascontiguousarray(a.reshape(RB, P).T)

    norms_f = fold(norms[:, 0])
    posf_f = fold(posf)

    in_maps = []
    # stream logits at fp16 (halves HBM traffic; global rel-err ~1e-4).
    # xv is taken from the QUANTIZED logits so the on-device delta at the
    # label column cancels the quantized bulk value exactly.
    lg16 = logits.astype(np.float16)
    xv = lg16[rows, np.clip(labels, 0, C - 1)].astype(np.float32)
    xv_f = fold(xv)
    ramp = np.ascontiguousarray(
        np.broadcast_to(np.arange(T, dtype=np.int16), (P, T))
    )
    for m in range(NCORES):
        c0 = m * CSH
        loc = labels - c0
        inr = (labels >= 0) & (loc >= 0) & (loc < CSH)
        locf = np.where(inr, loc, -1).astype(np.float32)
        shard = np.ascontiguousarray(lg16[:, c0 : c0 + CSH]).reshape(-1)
        sidecar = np.concatenate(
            [
                norms_f,
                posf_f,
                fold(inr.astype(np.float32)),
                fold(locf),
                xv_f,
            ],
            axis=1,
        )
        in_maps.append(
            {
                "logits": shard,
                "sidecar": np.ascontiguousarray(sidecar),
                "ramp": ramp,
            }
        )
    return in_maps


def kernel(logits, norms, labels, _trace=False, _trace_kwargs=None):
    from concourse import bass_utils

    nc = _get_program()
    in_maps = _prep_inputs(logits, norms, labels)
    res = bass_utils.run_bass_kernel_spmd(
        nc,
        in_maps,
        core_ids=list(range(NCORES)),
        trace=_trace,
        **(_trace_kwargs or {}),
    )
    _CACHED["last_result"] = res
    shards = [res.results[i]["out"].reshape(B, CSH) for i in range(NCORES)]
    return np.concatenate(shards, axis=1).astype(np.float32)
